# revision 17
# baseline (speedup 1.0000x reference)
"""Trainium2 Bass kernel for an LSTM-cell-like module.

Math (per the module):
    concat = [inputs, h]                 # [B, 2048]
    gates  = concat @ W + b              # [B, 3072]
    i, j, o = split(gates, 3, axis=1)
    i = sigmoid(i)
    new_c = (1 - i) * c + i * tanh(j)
    new_h = tanh(new_c) * sigmoid(o)
    returns (new_h, new_c)

Strategy: data-parallel across 8 NeuronCores on the batch dim (2048 rows
per core); W/b replicated.  Per core:
  - W is loaded once, cast fp32->bf16 in-flight (SWDGE cast DMA), and kept
    resident in SBUF ([2048, 3072] bf16 = 96KB/partition).
  - For each 128-row tile of the batch: x/h tiles are cast-loaded to bf16,
    transposed on the PE (features onto partitions), and used as the
    stationary operand of bf16 matmuls against the resident W
    (fp32 accumulation in PSUM).
  - bias is added with the vector engine (broadcast along partitions),
    sigmoid/tanh run on the scalar engine straight out of PSUM, and the
    LSTM elementwise runs on the vector engine in fp32.
"""

import sys

for _p in ("/opt/trn_rl_repo", "/opt/pypackages"):
    if _p not in sys.path:
        sys.path.append(_p)

import numpy as np

import concourse.mybir as mybir
import concourse.tile as tile
from concourse import bacc
from concourse.bass_utils import run_bass_kernel_spmd

N_CORES = 8
B = 16384
IN = 1024
H = 1024
K = IN + H                  # 2048 contraction dim
N3H = 3 * H                 # 3072 gate dim
P = 128                     # partitions
BC = B // N_CORES           # 2048 rows per core
M_TILES = BC // P           # 16 row-tiles per core
K_TILES = K // P            # 16 contraction tiles
N_CHUNK = 512               # matmul moving free dim (one PSUM bank)
N_CHUNKS = N3H // N_CHUNK   # 6

FP32 = mybir.dt.float32
BF16 = mybir.dt.bfloat16
AF = mybir.ActivationFunctionType


def build_nc(repeat: int = 1):
    """Build + compile the per-core Bass module (same program on all cores)."""
    nc = bacc.Bacc("TRN2", target_bir_lowering=False, debug=False,
                   num_devices=N_CORES)

    x_d = nc.dram_tensor("x", [BC, IN], FP32, kind="ExternalInput").ap()
    h_d = nc.dram_tensor("h", [BC, H], FP32, kind="ExternalInput").ap()
    c_d = nc.dram_tensor("c", [BC, H], FP32, kind="ExternalInput").ap()
    w_d = nc.dram_tensor("W", [K, N3H], FP32, kind="ExternalInput").ap()
    b_d = nc.dram_tensor("b", [N3H], FP32, kind="ExternalInput").ap()
    oh_d = nc.dram_tensor("out_h", [BC, H], FP32, kind="ExternalOutput").ap()
    oc_d = nc.dram_tensor("out_c", [BC, H], FP32, kind="ExternalOutput").ap()

    with tile.TileContext(nc) as tc:
        with (
            tc.tile_pool(name="wpool", bufs=K_TILES) as wpool,
            tc.tile_pool(name="consts", bufs=1) as consts,
            tc.tile_pool(name="xh", bufs=2) as xh_pool,
            tc.tile_pool(name="ct", bufs=32) as ct_pool,
            tc.tile_pool(name="cin", bufs=2) as c_pool,
            tc.tile_pool(name="gact", bufs=2) as gact_pool,
            tc.tile_pool(name="ew", bufs=2) as ew_pool,
            tc.tile_pool(name="psg", bufs=4, space="PSUM") as psg_pool,
        ):
            # ---- constants ----
            # bias broadcast to all partitions (stride-0 DMA read)
            b_bc = consts.tile([P, N3H], FP32)
            nc.scalar.dma_start(b_bc[:], b_d[None, :].to_broadcast((P, N3H)))

            # first row-tile's x/h ahead of the W stream in the SWDGE queue
            xb0 = xh_pool.tile([P, IN], BF16, tag="xb")
            nc.gpsimd.dma_start(xb0[:], x_d[0:P, :])
            hb0 = xh_pool.tile([P, H], BF16, tag="hb")
            nc.gpsimd.dma_start(hb0[:], h_d[0:P, :])

            if repeat == 0:
                # timing-baseline build: no compute, just a token DMA
                tok = xh_pool.tile([P, H], FP32, tag="tok")
                nc.sync.dma_start(tok[:], c_d[0:P, :])
                nc.sync.dma_start(oh_d[0:P, :], tok[:])
                nc.sync.dma_start(oc_d[0:P, :], tok[:])

            # ---- resident bf16 weights (cast in-flight by SWDGE) ----
            w_sb = []
            for k in range(K_TILES if repeat else 0):
                wt = wpool.tile([P, N3H], BF16, tag="w")
                nc.gpsimd.dma_start(wt[:], w_d[k * P:(k + 1) * P, :])
                w_sb.append(wt)

            for _ in range(repeat):
                for m in range(M_TILES):
                    rows = slice(m * P, (m + 1) * P)
                    # cast-load x/h row-tile to bf16 (m==0 preloaded above)
                    if m == 0 and xb0 is not None:
                        xb, hb = xb0, hb0
                        xb0 = hb0 = None
                    else:
                        xb = xh_pool.tile([P, IN], BF16, tag="xb")
                        nc.gpsimd.dma_start(xb[:], x_d[rows, :])
                        hb = xh_pool.tile([P, H], BF16, tag="hb")
                        nc.gpsimd.dma_start(hb[:], h_d[rows, :])

                    # transpose to concatT tiles (features on partitions)
                    # via the DMA XBAR, SBUF->SBUF on the sync HWDGE ring
                    ct = []
                    for kk in range(K_TILES):
                        src = xb if kk < IN // P else hb
                        col = (kk % (IN // P)) * P
                        st = ct_pool.tile([P, P], BF16, tag="ct")
                        nc.sync.dma_start(st[:], src[:, col:col + P],
                                          transpose=True)
                        ct.append(st)

                    c_t = c_pool.tile([P, H], FP32, tag="c")
                    nc.scalar.dma_start(c_t[:], c_d[rows, :])

                    # gates: 3 psum tiles (i, j, o), each [128, 1024] fp32
                    pg = [psg_pool.tile([P, 2 * N_CHUNK], FP32, tag="pg",
                                        name=f"pg{g}")
                          for g in range(3)]
                    # n-outer so early chunks complete (and drain) while
                    # later chunks still accumulate
                    for n in range(N_CHUNKS):
                        dst = pg[n // 2][:, (n % 2) * N_CHUNK:
                                         (n % 2) * N_CHUNK + N_CHUNK]
                        for k in range(K_TILES):
                            nc.tensor.matmul(
                                dst,
                                lhsT=ct[k][:],
                                rhs=w_sb[k][:, n * N_CHUNK:(n + 1) * N_CHUNK],
                                start=(k == 0),
                                stop=(k == K_TILES - 1),
                            )
                        if n % 2 == 1:
                            # bias for the finished [128, 1024] gate chunk,
                            # in-place on PSUM (broadcast along free dim)
                            g = n // 2
                            nc.vector.tensor_add(
                                pg[g][:], pg[g][:], b_bc[:, g * H:(g + 1) * H])

                    ig = gact_pool.tile([P, H], FP32, tag="ig")
                    nc.scalar.activation(ig[:], pg[0][:], AF.Sigmoid)
                    tj = gact_pool.tile([P, H], FP32, tag="tj")
                    nc.scalar.activation(tj[:], pg[1][:], AF.Tanh)
                    og = gact_pool.tile([P, H], FP32, tag="og")
                    nc.scalar.activation(og[:], pg[2][:], AF.Sigmoid)

                    # new_c = c + i*(tanh(j) - c);  new_h = tanh(new_c)*sig(o)
                    d = ew_pool.tile([P, H], FP32, tag="d")
                    nc.vector.tensor_sub(d[:], tj[:], c_t[:])
                    nc.vector.tensor_mul(d[:], ig[:], d[:])
                    cnew = ew_pool.tile([P, H], FP32, tag="cnew")
                    nc.vector.tensor_add(cnew[:], d[:], c_t[:])
                    th = ew_pool.tile([P, H], FP32, tag="th")
                    nc.scalar.activation(th[:], cnew[:], AF.Tanh)
                    hnew = ew_pool.tile([P, H], FP32, tag="hnew")
                    nc.vector.tensor_mul(hnew[:], th[:], og[:])

                    nc.scalar.dma_start(oc_d[rows, :], cnew[:])
                    nc.scalar.dma_start(oh_d[rows, :], hnew[:])

    nc.compile()
    return nc


_NC_CACHE = {}


def _get_nc(repeat: int = 1):
    if repeat not in _NC_CACHE:
        _NC_CACHE[repeat] = build_nc(repeat)
    return _NC_CACHE[repeat]


def kernel(inputs, h, c, W, b):
    inputs = np.ascontiguousarray(np.asarray(inputs, dtype=np.float32))
    h = np.ascontiguousarray(np.asarray(h, dtype=np.float32))
    c = np.ascontiguousarray(np.asarray(c, dtype=np.float32))
    W = np.ascontiguousarray(np.asarray(W, dtype=np.float32))
    b = np.ascontiguousarray(np.asarray(b, dtype=np.float32))

    nc = _get_nc()
    in_maps = []
    for i in range(N_CORES):
        rows = slice(i * BC, (i + 1) * BC)
        in_maps.append({
            "x": inputs[rows], "h": h[rows], "c": c[rows], "W": W, "b": b,
        })
    res = run_bass_kernel_spmd(nc, in_maps, core_ids=list(range(N_CORES)))
    new_h = np.concatenate([res.results[i]["out_h"] for i in range(N_CORES)], 0)
    new_c = np.concatenate([res.results[i]["out_c"] for i in range(N_CORES)], 0)
    return new_h, new_c
